# revision 8
# baseline (speedup 1.0000x reference)
"""Single-head causal attention on 8 TRN2 NeuronCores.

Problem: nn_AttentionHead (B=8, S=2048, D_MODEL=2048, HEAD_DIM=128), f32.
Sharding: data-parallel over batch -- one batch element per core, no
collectives.

Per-core algorithm (batch element b = core id):
  x        = hidden_state[b]            [S, D]   (bf16, d-chunk-major layout)
  xT tiles = DMA-transpose loads        [D, S]   8 x [128, 4096] in SBUF
  qT = (Wq/sqrt(H)).T @ x.T + bq'       [H, S]   (scale folded into Wq)
  kT = Wk.T @ x.T + bk                  [H, S]
  vT = Wv.T @ x.T + bv                  [H, S]   -> v via 16 PE transposes
  scoresT_j = kT_j.T @ qT               [sk=128, sq>=j*128]  causal blocks only
  expT_j = exp(scoresT_j + diag mask)   bf16, feeds AV matmul as lhsT
  out_i = sum_j expT_j(block i).T @ [v_j | 1]   -> [sq=128, H+1]
  out   = out_i[:, :H] / out_i[:, H]    (ones column = softmax denominator)

All matmuls bf16 (PSUM accumulates f32).  No max-subtraction in softmax:
scores ~ N(0,1) so exp() cannot overflow f32.  q/k accumulate chunk-by-chunk
under the (serial-xbar) transpose wall; all DMAs stay on one HWDGE ring
(concurrent copy||transpose across rings corrupts via the xbar-mode hazard).
"""

import sys

for _p in ("/opt/trn_rl_repo", "/opt/trn_rl_repo/concourse"):
    if _p not in sys.path:
        sys.path.insert(0, _p)

import ml_dtypes
import numpy as np

B, S, D, H = 8, 2048, 2048, 128
P = 128                 # partition size
DC = D // P             # d-chunks (16)
NT = S // P             # s-tiles (16)
NEG = -1.0e9
N_CORES = 8

BF16 = ml_dtypes.bfloat16


def build_graph():
    import concourse.bass as bass
    import concourse.mybir as mybir
    import concourse.tile as tile
    from concourse import bacc

    f32 = mybir.dt.float32
    bf16 = mybir.dt.bfloat16
    Exp = mybir.ActivationFunctionType.Exp

    nc = bacc.Bacc("TRN2", target_bir_lowering=False, debug=False)

    # x in d-chunk-major layout, two chunks per transpose call:
    # x_ext[g, i*S+s, p] = x[s, (2g+i)*128+p]
    x_ext = nc.declare_dram_parameter("x", [DC // 2, 2 * S, P], bf16, isOutput=False)
    # weights pre-arranged host-side to [P, DC*H]: w_ext[p, c*H+h] = W[c*128+p, h]
    wq_ext = nc.declare_dram_parameter("wq", [P, DC * H], bf16, isOutput=False)
    wk_ext = nc.declare_dram_parameter("wk", [P, DC * H], bf16, isOutput=False)
    wv_ext = nc.declare_dram_parameter("wv", [P, DC * H], bf16, isOutput=False)
    bq_ext = nc.declare_dram_parameter("bq", [H], f32, isOutput=False)
    bk_ext = nc.declare_dram_parameter("bk", [H], f32, isOutput=False)
    bv_ext = nc.declare_dram_parameter("bv", [H], f32, isOutput=False)
    mask_ext = nc.declare_dram_parameter("mask", [P, P], f32, isOutput=False)
    ident_ext = nc.declare_dram_parameter("ident", [P, P], bf16, isOutput=False)
    out_ext = nc.declare_dram_parameter("out", [S, H], f32, isOutput=True)
    out_r = out_ext.rearrange("(i p) h -> p i h", p=P)

    with tile.TileContext(nc) as tc:
        with (
            tc.tile_pool(name="xt", bufs=1) as xt_pool,
            tc.tile_pool(name="wts", bufs=1) as w_pool,
            tc.tile_pool(name="qk", bufs=1) as qk_pool,
            tc.tile_pool(name="vp", bufs=1) as v_pool,
            tc.tile_pool(name="et", bufs=1) as e_pool,
            tc.tile_pool(name="ob", bufs=1) as o_pool,
            tc.tile_pool(name="sm", bufs=4) as small_pool,
        ):
            # ---- constant / weight loads (single HWDGE ring, before the
            # transposes -- exactly one copy->transpose xbar transition) --
            wq_sb = w_pool.tile([P, DC, H], bf16, tag="wq")
            wk_sb = w_pool.tile([P, DC, H], bf16, tag="wk")
            wv_sb = w_pool.tile([P, DC, H], bf16, tag="wv")
            bq_sb = w_pool.tile([P, 1], f32, tag="bq")
            bk_sb = w_pool.tile([P, 1], f32, tag="bk")
            bv_sb = w_pool.tile([P, 1], f32, tag="bv")
            mask_sb = w_pool.tile([P, P], f32, tag="mask")
            ident_sb = w_pool.tile([P, P], bf16, tag="ident")
            nc.sync.dma_start(wq_sb[:], wq_ext.rearrange("p (c h) -> p c h", h=H))
            nc.sync.dma_start(bq_sb[:], bq_ext.rearrange("(p o) -> p o", o=1))
            nc.sync.dma_start(wk_sb[:], wk_ext.rearrange("p (c h) -> p c h", h=H))
            nc.sync.dma_start(bk_sb[:], bk_ext.rearrange("(p o) -> p o", o=1))
            nc.sync.dma_start(wv_sb[:], wv_ext.rearrange("p (c h) -> p c h", h=H))
            nc.sync.dma_start(bv_sb[:], bv_ext.rearrange("(p o) -> p o", o=1))
            nc.sync.dma_start(mask_sb[:], mask_ext[:])
            nc.sync.dma_start(ident_sb[:], ident_ext[:])

            # ---- x.T via 8 double-chunk DMA transposes -----------------
            xt2 = []
            for g in range(DC // 2):
                t = xt_pool.tile([P, 2 * S], bf16, tag=f"xt{g}", name=f"xt{g}")
                nc.sync.dma_start(t[:], x_ext[g], transpose=True)
                xt2.append(t)

            def xtv(c, lo, width):
                return xt2[c // 2][:, (c % 2) * S + lo : (c % 2) * S + lo + width]

            # ---- q+k projections, c-streaming under the transposes -----
            kT_sb = qk_pool.tile([P, S], bf16, tag="kT")
            qT_sb = qk_pool.tile([P, S], bf16, tag="qT")
            with tc.tile_pool(name="pqk", bufs=1, space="PSUM") as pp_qk:
                qkps = [
                    pp_qk.tile([P, 512], f32, tag=f"qkps{i}", name=f"qkps{i}")
                    for i in range(8)
                ]
                for c in range(DC):
                    for n in range(4):
                        nc.tensor.matmul(
                            qkps[n][:],
                            wq_sb[:, c, :],
                            xtv(c, n * 512, 512),
                            start=(c == 0),
                            stop=(c == DC - 1),
                        )
                        nc.tensor.matmul(
                            qkps[4 + n][:],
                            wk_sb[:, c, :],
                            xtv(c, n * 512, 512),
                            start=(c == 0),
                            stop=(c == DC - 1),
                        )
                # kT chunk 0 first: scores_0 needs it plus all of qT
                nc.vector.tensor_scalar_add(kT_sb[:, 0:512], qkps[4][:], bk_sb[:])
                for n in range(4):
                    nc.vector.tensor_scalar_add(
                        qT_sb[:, n * 512 : (n + 1) * 512], qkps[n][:], bq_sb[:]
                    )
                for n in range(1, 4):
                    nc.vector.tensor_scalar_add(
                        kT_sb[:, n * 512 : (n + 1) * 512], qkps[4 + n][:], bk_sb[:]
                    )

            # ---- vT projection (wide matmuls, bias per-partition) ------
            vT_sb = v_pool.tile([P, S], bf16, tag="vT")
            with tc.tile_pool(name="pvt", bufs=2, space="PSUM") as pp_vt:
                for n in range(4):
                    ps = pp_vt.tile([P, 512], f32, tag="vtps")
                    for c in range(DC):
                        nc.tensor.matmul(
                            ps[:],
                            wv_sb[:, c, :],
                            xtv(c, n * 512, 512),
                            start=(c == 0),
                            stop=(c == DC - 1),
                        )
                    nc.vector.tensor_scalar_add(
                        vT_sb[:, n * 512 : (n + 1) * 512], ps[:], bv_sb[:]
                    )

            # ---- phase 2 pools: scores(4) + vtr(2) + out(2) = 8 banks --
            v_sb = v_pool.tile([P, NT, H + 1], bf16, tag="v")
            nc.vector.memset(v_sb[:, :, H], 1.0)
            out_sb = o_pool.tile([P, NT, H], f32, tag="out")
            expT = [None] * NT

            with (
                tc.tile_pool(name="pss", bufs=2, space="PSUM") as pp_s,
                tc.tile_pool(name="ptr", bufs=2, space="PSUM") as pp_t,
                tc.tile_pool(name="pso", bufs=2, space="PSUM") as pp_o,
            ):
                # v blocks via PE transpose (j-ascending so AV_j unblocks)
                for j in range(NT):
                    ps_t = pp_t.tile([P, P], bf16, tag="tps")
                    nc.tensor.transpose(
                        ps_t[:], vT_sb[:, j * P : (j + 1) * P], ident_sb[:]
                    )
                    nc.vector.tensor_copy(v_sb[:, j, 0:H], ps_t[:])

                for j in range(NT):
                    # causal scoresT_j + exp (1024-wide psum, fewer ACT ops)
                    width = (NT - j) * P
                    et = e_pool.tile(
                        [P, width], bf16, tag=f"expT{j}", name=f"expT{j}"
                    )
                    expT[j] = et
                    off = 0
                    while off < width:
                        w = min(1024, width - off)
                        ps_s = pp_s.tile([P, 1024], f32, tag="sps")
                        for o2 in range(0, w, 512):
                            w2 = min(512, w - o2)
                            nc.tensor.matmul(
                                ps_s[:, o2 : o2 + w2],
                                kT_sb[:, j * P : (j + 1) * P],
                                qT_sb[:, j * P + off + o2 : j * P + off + o2 + w2],
                                start=True,
                                stop=True,
                            )
                        if off == 0:
                            nc.vector.tensor_add(
                                ps_s[:, 0:P], ps_s[:, 0:P], mask_sb[:]
                            )
                        nc.scalar.activation(
                            et[:, off : off + w], ps_s[:, 0:w], Exp
                        )
                        off += w

                    # AV row i=j (expT_0..j and v_0..j are all ready)
                    i = j
                    ps_o = pp_o.tile([P, H + 1], f32, tag="ops")
                    for jj in range(i + 1):
                        nc.tensor.matmul(
                            ps_o[:],
                            expT[jj][:, (i - jj) * P : (i - jj + 1) * P],
                            v_sb[:, jj, :],
                            start=(jj == 0),
                            stop=(jj == i),
                        )
                    recip = small_pool.tile([P, 1], f32, tag="recip")
                    nc.vector.reciprocal(recip[:], ps_o[:, H : H + 1])
                    nc.vector.tensor_scalar_mul(
                        out_sb[:, i, :], ps_o[:, 0:H], recip[:]
                    )
                    if i % 4 == 3:
                        nc.sync.dma_start(
                            out_r[:, i - 3 : i + 1, :],
                            out_sb[:, i - 3 : i + 1, :],
                        )

    nc.compile()
    return nc


_cached = {}


def _get_graph():
    if "nc" not in _cached:
        _cached["nc"] = build_graph()
    return _cached["nc"]


def _prep_inputs(hidden_state, Wq, bq, Wk, bk, Wv, bv):
    hs = np.asarray(hidden_state, dtype=np.float32)
    scale = np.float32(1.0 / np.sqrt(np.float32(H)))

    def prep_w(w, s=None):
        w = np.asarray(w, dtype=np.float32)
        if s is not None:
            w = w * s
        # [D, H] -> [P, DC*H] with w_out[p, c*H+h] = w[c*P+p, h]
        return np.ascontiguousarray(
            w.reshape(DC, P, H).transpose(1, 0, 2).reshape(P, DC * H)
        ).astype(BF16)

    wq = prep_w(Wq, scale)
    wk = prep_w(Wk)
    wv = prep_w(Wv)
    bq_s = (np.asarray(bq, dtype=np.float32) * scale).astype(np.float32)
    bk_f = np.asarray(bk, dtype=np.float32)
    bv_f = np.asarray(bv, dtype=np.float32)
    r = np.arange(P)
    mask = np.where(r[:, None] > r[None, :], np.float32(NEG), np.float32(0.0)).astype(
        np.float32
    )
    ident = np.eye(P, dtype=np.float32).astype(BF16)

    in_maps = []
    for b in range(N_CORES):
        # x -> d-chunk-major [DC//2, 2S, P] so transpose DMAs read contiguously
        xb = np.ascontiguousarray(
            hs[b].astype(BF16).reshape(S, DC, P).transpose(1, 0, 2)
        ).reshape(DC // 2, 2 * S, P)
        in_maps.append(
            {
                "x": xb,
                "wq": wq,
                "wk": wk,
                "wv": wv,
                "bq": bq_s,
                "bk": bk_f,
                "bv": bv_f,
                "mask": mask,
                "ident": ident,
            }
        )
    return in_maps


def kernel(hidden_state, Wq, bq, Wk, bk, Wv, bv):
    from concourse.bass_utils import run_bass_kernel_spmd

    in_maps = _prep_inputs(hidden_state, Wq, bq, Wk, bk, Wv, bv)
    nc = _get_graph()
    res = run_bass_kernel_spmd(nc, in_maps, core_ids=list(range(N_CORES)))
    out = np.stack([res.results[i]["out"] for i in range(N_CORES)], axis=0)
    return out.astype(np.float32)


def run_traced(hidden_state, Wq, bq, Wk, bk, Wv, bv):
    """Like kernel() but with NTFF tracing; returns (out, BassKernelResults)."""
    from concourse.bass_utils import run_bass_kernel_spmd

    in_maps = _prep_inputs(hidden_state, Wq, bq, Wk, bk, Wv, bv)
    nc = _get_graph()
    res = run_bass_kernel_spmd(nc, in_maps, core_ids=list(range(N_CORES)), trace=True)
    out = np.stack([res.results[i]["out"] for i in range(N_CORES)], axis=0).astype(
        np.float32
    )
    return out, res
